# revision 5
# baseline (speedup 1.0000x reference)
"""MDTA (Restormer transposed attention) Bass kernel for 8x TRN2 cores.

Per core = one batch image, data-parallel over B=8. Engine-balanced design
(v1 ran the depthwise conv as diagonal matmuls and was PE-bound at ~455us
busy; this version sims at ~344us):

  * depthwise 3x3 split per (slab, chunk, 8-row half) across engines:
    PE keeps ~half (diagonal matmuls, chunks m0/m1), DVE runs
    mul(tensor_scalar@4x) + add(tensor_tensor@2x) chains, and one half per
    slab is split DVE/Pool ('dp': DVE taps 0-4, Pool taps 5-8, DVE merge).
    Pool cannot run scalar_tensor_tensor or touch PSUM (ISA limits), which
    caps its share.
  * the 64-channel tail chunk (v ch 128:192) runs on PE with both spatial
    halves packed into one 128-row diagonal matmul.
  * q/k L2-norms via Act Square+accum_out per slab (keeps the gram to a
    single qT k matmul per head-pair per 128 columns).
  * attn@v and the 1x1 proj fused: out = (W_proj @ A) @ v with A the
    block-diagonal softmax matrix — saves 33k PE cycles and a PSUM evac.
  * DMA transposes batched: one instruction per [128, 16*128] chunk-slab
    (16 xbar 16x128 tiles each) feeding the gram with [x, ch] tiles.
  * all big matmul operands bf16 (fp8 blows the 2e-2 error budget: pw in
    fp8e4m3 alone measured 4.2e-2), PSUM accumulation fp32, depthwise
    accumulator fp16.
"""
import sys

for _p in ("/opt/trn_rl_repo", "/root/.axon_site/_ro/trn_rl_repo"):
    if _p not in sys.path:
        sys.path.insert(0, _p)

import numpy as np
import ml_dtypes  # noqa: F401

import concourse.bass as bass
from concourse import bacc, mybir
import concourse.tile as tile
from concourse.bass_utils import run_bass_kernel_spmd

F16 = mybir.dt.bfloat16
FP16 = mybir.dt.float16
F32 = mybir.dt.float32
ADD = mybir.AluOpType.add
MULT = mybir.AluOpType.mult

B, C, HH, WW = 8, 192, 128, 128
N = HH * WW               # 16384
HEADS, HD = 4, 48
C3 = 3 * C                # 576
RSLAB = 16
NSLAB = HH // RSLAB       # 8
NCH = 4                   # full 128-channel chunks m0..m3
TAPS = [(dy, dx) for dy in range(3) for dx in range(3)]

# dw engine assignment per (slab, m): (top-half engine, bottom-half engine),
# each half = 8 output rows. 'pe' only on m0/m1 (wdiag resident for those).
_pat = [[("pe", "pe"), ("pe", "pe"), ("dp", "dve"), ("dve", "dve")],
        [("pe", "pe"), ("pe", "pe"), ("dp", "dve"), ("dve", "dve")],
        [("pe", "pe"), ("pe", "pe"), ("dp", "dve"), ("dve", "dve")],
        [("pe", "pe"), ("pe", "pe"), ("dp", "dve"), ("dve", "dve")],
        [("pe", "pe"), ("pe", "pe"), ("dp", "dve"), ("dve", "dve")],
        [("pe", "pe"), ("pe", "pe"), ("dp", "dve"), ("dve", "dve")],
        [("pe", "pe"), ("pe", "pe"), ("dp", "dve"), ("dve", "dve")],
        [("pe", "pe"), ("pe", "pe"), ("dp", "dve"), ("dve", "dve")]]
DW_ASSIGN = {(_s, _m): _pat[_s][_m] for _s in range(NSLAB) for _m in range(NCH)}
# engine for pw-PSUM evac per chunk m0..m4 (Act keeps pace w/ PE best)
EVAC_ENG = ["act", "act", "act", "act", "act"]

_CACHE = {}


def build_program(inv_temp: float):
    nc = bacc.Bacc("TRN2", target_bir_lowering=False, debug=False, num_devices=8)

    x16 = nc.dram_tensor("x16", [C, HH, WW], F16, kind="ExternalInput").ap()
    wpwa_d = nc.dram_tensor("wpwa", [128, C3], F16, kind="ExternalInput").ap()
    wpwb_d = nc.dram_tensor("wpwb", [64, C3], F16, kind="ExternalInput").ap()
    wsc_d = nc.dram_tensor("wsc", [128, NCH, 9], F32, kind="ExternalInput").ap()
    wdg_d = nc.dram_tensor("wdiag", [128, 2, 9, 128], F16, kind="ExternalInput").ap()
    wd4_d = nc.dram_tensor("wdiag4", [128, 9, 128], F16, kind="ExternalInput").ap()
    wjlo_d = nc.dram_tensor("wpjTlo", [96, C], F16, kind="ExternalInput").ap()
    wjhi_d = nc.dram_tensor("wpjThi", [96, C], F16, kind="ExternalInput").ap()
    id32_d = nc.dram_tensor("ident32", [96, 96], F32, kind="ExternalInput").ap()
    bmask_d = nc.dram_tensor("bmask", [96, 96], F32, kind="ExternalInput").ap()
    out_d = nc.dram_tensor("out", [C, N], F32, kind="ExternalOutput").ap()

    from contextlib import ExitStack
    with tile.TileContext(nc) as tc:
        with tc.tile_pool(name="res", bufs=1) as res, \
             tc.tile_pool(name="sm", bufs=1) as sm:
            p1 = ExitStack()
            xp = p1.enter_context(tc.tile_pool(name="xp", bufs=2))
            qpre = p1.enter_context(tc.tile_pool(name="qpre", bufs=2))
            qkp = p1.enter_context(tc.tile_pool(name="qk", bufs=2))
            qktp = p1.enter_context(tc.tile_pool(name="qkt", bufs=2))
            accp = p1.enter_context(tc.tile_pool(name="accp", bufs=1))
            prodp = p1.enter_context(tc.tile_pool(name="prodp", bufs=1))
            pwps = p1.enter_context(tc.tile_pool(name="pwps", bufs=3, space="PSUM"))
            dwps = p1.enter_context(tc.tile_pool(name="dwps", bufs=2, space="PSUM"))
            dw4ps = p1.enter_context(tc.tile_pool(name="dw4ps", bufs=1, space="PSUM"))
            gps = p1.enter_context(tc.tile_pool(name="gps", bufs=1, space="PSUM"))

            # --- resident weights ---
            wpa = res.tile([128, C3], F16, tag="wpa")
            wpb = res.tile([64, C3], F16, tag="wpb")
            wsc = res.tile([128, NCH, 9], F32, tag="wsc")
            wdg = res.tile([128, 2, 9, 128], F16, tag="wdg")
            wd4 = res.tile([128, 9, 128], F16, tag="wd4")
            wjlo = res.tile([96, C], F16, tag="wjlo")
            wjhi = res.tile([96, C], F16, tag="wjhi")
            id32 = res.tile([96, 96], F32, tag="id32")
            bmask = res.tile([96, 96], F32, tag="bmask")
            for t, d in ((wpa, wpwa_d), (wpb, wpwb_d), (wsc, wsc_d), (wdg, wdg_d),
                         (wd4, wd4_d), (wjlo, wjlo_d), (wjhi, wjhi_d),
                         (id32, id32_d), (bmask, bmask_d)):
                nc.sync.dma_start(t[:], d[:])

            # v resident (bf16): v ch 0:128 -> v16a, v ch 128:192 -> v16b
            v16a = res.tile([128, N], F16, tag="v16a")
            v16b = res.tile([64, N], F16, tag="v16b")
            # q,k norm partials per chunk: [128, NSLAB]
            nrmp = [res.tile([128, NSLAB], F32, tag=f"nrmp{m}", name=f"nrmp{m}") for m in range(3)]
            nsc = res.tile([128, RSLAB * 128], F16, tag="nsc")  # norm scratch
            # gram accumulators: [96, 96] qT k per head-pair
            g_t = [gps.tile([96, 96], F32, tag=f"g{p}", name=f"g{p}")[:]
                   for p in range(2)]

            def emit_gram(qkts, s_of):
                for rr in range(RSLAB):
                    first = s_of == 0 and rr == 0
                    last = s_of == NSLAB - 1 and rr == RSLAB - 1
                    for p in range(2):
                        qpair = qkts[:, rr, 96 * p:96 * p + 96]
                        kpair = qkts[:, rr, 192 + 96 * p:288 + 96 * p]
                        nc.tensor.matmul(g_t[p], qpair, kpair,
                                         start=first, stop=last, skip_group_check=True)

            # ---------------- pass 1: slab pipeline ----------------
            def x_load(s):
                jlo = 1 if s == 0 else 0
                jhi = RSLAB + 1 if s == NSLAB - 1 else RSLAB + 2
                r0 = RSLAB * s
                xa = xp.tile([128, RSLAB + 2, 128], F16, tag="xa", name=f"xa_{s}")
                xb = xp.tile([64, RSLAB + 2, 128], F16, tag="xb", name=f"xb_{s}")
                nc.sync.dma_start(xa[:, jlo:jhi, :],
                                  x16[0:128, r0 - 1 + jlo:r0 - 1 + jhi, :])
                nc.sync.dma_start(xb[:, jlo:jhi, :],
                                  x16[128:192, r0 - 1 + jlo:r0 - 1 + jhi, :])
                return xa, xb

            pend = None
            prev_qk = None
            x_cur = x_load(0)
            for s in range(NSLAB):
                r0 = RSLAB * s
                jlo = 1 if s == 0 else 0             # first valid row in 18-row window
                jhi = RSLAB + 1 if s == NSLAB - 1 else RSLAB + 2

                xa, xb = x_cur
                if s + 1 < NSLAB:
                    x_cur = x_load(s + 1)            # prefetch next slab's input

                # qp tiles: m0..m3 [128, 18, 130]; m4 paired [128, 18, 66]
                qp = [qpre.tile([128, RSLAB + 2, 130], F16, tag=f"qp{m}",
                                name=f"qp{m}_{s}") for m in range(NCH)]
                qp4 = qpre.tile([128, RSLAB + 2, 66], F16, tag="qp4", name=f"qp4_{s}")
                for m in range(NCH):
                    if s < 2:   # ring bufs: halo cols stay zero after first use
                        nc.gpsimd.memset(qp[m][:, :, 0:1], 0.0)
                        nc.gpsimd.memset(qp[m][:, :, 129:130], 0.0)
                    if s == 0:
                        nc.gpsimd.memset(qp[m][:, 0:1, :], 0.0)
                    if s == NSLAB - 1:
                        nc.gpsimd.memset(qp[m][:, RSLAB + 1:RSLAB + 2, :], 0.0)
                if s < 2:
                    nc.gpsimd.memset(qp4[0:64, :, 0:1], 0.0)      # x=-1 (left)
                    nc.gpsimd.memset(qp4[64:128, :, 65:66], 0.0)  # x=128 (right)
                if s == 0:
                    nc.gpsimd.memset(qp4[:, 0:1, :], 0.0)
                if s == NSLAB - 1:
                    nc.gpsimd.memset(qp4[:, RSLAB + 1:RSLAB + 2, :], 0.0)

                # pointwise conv: 8-row psum tiles [cs, 2, 512] (2 banks),
                # filled by 4-row matmul pairs, drained by ONE evac instr.
                groups = []
                j = jlo
                while j < jhi:
                    groups.append((j, min(j + 4, jhi)))
                    j = min(j + 4, jhi)
                for m in (2, 3, 0, 1, 4):
                    cs = 128 if m < NCH else 64
                    c0 = 128 * m
                    for (ja, jb) in groups:
                        nr = jb - ja
                        ps = pwps.tile([cs, 1, 512], F32, tag="pw",
                                       name=f"pw_{s}_{m}_{ja}")
                        dst = ps[:, 0, 0:128 * nr]
                        nc.tensor.matmul(dst, wpa[:, c0:c0 + cs],
                                         xa[:, ja:jb, :], start=True, stop=False)
                        nc.tensor.matmul(dst, wpb[:, c0:c0 + cs],
                                         xb[:, ja:jb, :], start=False, stop=True)
                        psr = ps.rearrange("p a (r x) -> p (a r) x", x=128)
                        if m < NCH:
                            nc.scalar.copy(qp[m][:, ja:jb, 1:129], psr[:, 0:nr, :])
                        else:
                            # m4: paired evac (left cols -1..64, right 63..128)
                            nc.scalar.copy(qp4[0:64, ja:jb, 1:66],
                                           psr[0:64, 0:nr, 0:65])
                            nc.scalar.copy(qp4[64:128, ja:jb, 0:65],
                                           psr[0:64, 0:nr, 63:128])

                # ---- gram for previous slab (PE; transposes long done) ----
                if pend is not None:
                    emit_gram(pend, s - 1)
                    pend = None

                # ---- depthwise for m0..m3, half-chunks on assigned engines ----
                qk = [qkp.tile([128, RSLAB, 128], F16, tag=f"qk{m}",
                               name=f"qk{m}_{s}") for m in range(3)]
                HR = RSLAB // 2   # rows per half
                for m in range(NCH):
                    for h, eng in enumerate(DW_ASSIGN[(s, m)]):
                        rlo = HR * h
                        if m < 3:
                            dst = qk[m][:, rlo:rlo + HR, :]
                        else:
                            dst = v16a.rearrange(
                                "p (s r x) -> p s r x",
                                s=NSLAB, r=RSLAB)[:, s, rlo:rlo + HR, :]
                        if eng == "pe":
                            for g in range(2):   # 4-row groups -> 1 psum bank
                                g0 = rlo + 4 * g
                                dps = dwps.tile([128, 4, 128], F32, tag="dwpe",
                                                name=f"dwpe_{s}_{m}_{h}_{g}")
                                for t, (dy, dx) in enumerate(TAPS):
                                    nc.tensor.matmul(
                                        dps[:], wdg[:, m, t, :],
                                        qp[m][:, g0 + dy:g0 + dy + 4, dx:dx + 128],
                                        start=(t == 0), stop=(t == 8))
                                nc.scalar.copy(qk[m][:, g0:g0 + 4, :], dps[:])
                        elif eng in ("dve", "act"):
                            acc = accp.tile([128, HR, 128], FP16, tag="acc_dve",
                                            name=f"acc_{s}_{m}_{h}")
                            prod = prodp.tile([128, HR, 128], FP16, tag="prod",
                                              name=f"prod_{s}_{m}_{h}")
                            def mul(dst_t, src_t, w_t):
                                if eng == "act":
                                    nc.scalar.activation(
                                        dst_t, src_t,
                                        mybir.ActivationFunctionType.Copy,
                                        scale=w_t)
                                else:
                                    nc.vector.tensor_scalar_mul(dst_t, src_t, w_t)
                            for t, (dy, dx) in enumerate(TAPS):
                                src = qp[m][:, rlo + dy:rlo + dy + HR, dx:dx + 128]
                                w_ap = wsc[:, m, t:t + 1]
                                if t == 0:
                                    mul(acc[:], src, w_ap)
                                elif t < 8:
                                    mul(prod[:], src, w_ap)
                                    nc.vector.tensor_tensor(acc[:], acc[:], prod[:], op=ADD)
                                else:
                                    mul(prod[:], src, w_ap)
                                    nc.vector.tensor_tensor(dst, acc[:], prod[:], op=ADD)
                        else:  # 'dp': DVE taps 0-4 + Pool taps 5-8, DVE merge
                            accd = accp.tile([128, HR, 128], FP16, tag="acc_dve",
                                             name=f"accd_{s}_{m}_{h}")
                            accq = accp.tile([128, HR, 128], FP16, tag="acc_pool",
                                             name=f"accq_{s}_{m}_{h}")
                            prod = prodp.tile([128, HR, 128], FP16, tag="prod",
                                              name=f"prodd_{s}_{m}_{h}")
                            prodq = prodp.tile([128, HR, 128], FP16, tag="prodq",
                                               name=f"prodq_{s}_{m}_{h}")
                            for t, (dy, dx) in enumerate(TAPS):
                                src = qp[m][:, rlo + dy:rlo + dy + HR, dx:dx + 128]
                                w_ap = wsc[:, m, t:t + 1]
                                if t == 0:
                                    nc.vector.tensor_scalar_mul(accd[:], src, w_ap)
                                elif t < 5:
                                    nc.vector.tensor_scalar_mul(prod[:], src, w_ap)
                                    nc.vector.tensor_tensor(accd[:], accd[:], prod[:], op=ADD)
                                elif t == 5:
                                    nc.gpsimd.tensor_scalar_mul(accq[:], src, w_ap)
                                else:
                                    nc.gpsimd.tensor_scalar_mul(prodq[:], src, w_ap)
                                    nc.gpsimd.tensor_tensor(accq[:], accq[:], prodq[:], op=ADD)
                            nc.vector.tensor_tensor(dst, accd[:], accq[:], op=ADD)

                # ---- m4 depthwise on PE (spatially paired); evacs on Pool ----
                for g in range(2):   # 8-row groups
                    dps = dw4ps.tile([128, 8, 64], F32, tag="dw4", name=f"dw4_{s}_{g}")
                    for t, (dy, dx) in enumerate(TAPS):
                        nc.tensor.matmul(
                            dps[:], wd4[:, t, :],
                            qp4[:, 8 * g + dy:8 * g + dy + 8, dx:dx + 64],
                            start=(t == 0), stop=(t == 8))
                    vv = v16b.rearrange("p (b r x) -> p b r x", r=8, x=128)
                    bi = (r0 * 128 + 1024 * g) // 1024
                    nc.scalar.copy(vv[:, bi, :, 0:64], dps[0:64, :, :])
                    nc.scalar.copy(vv[:, bi, :, 64:128], dps[64:128, :, :])

                # ---- q,k norms for PREVIOUS slab (Act; deps long ready) ----
                if prev_qk is not None:
                    for m in range(3):
                        nc.scalar.activation(
                            nsc.rearrange("p (r x) -> p r x", r=RSLAB)[:],
                            prev_qk[m][:],
                            mybir.ActivationFunctionType.Square,
                            accum_out=nrmp[m][:, s - 1:s])

                # ---- batched DMA transposes -> [x, rr, ch] tiles ----
                qkt = qktp.tile([128, RSLAB, 384], F16, tag="qkt", name=f"qkt_{s}")
                for m in range(3):
                    nc.sync.dma_start_transpose(
                        qkt[:, :, 128 * m:128 * m + 128],
                        qk[m].rearrange("p a b -> p (a b)"))

                pend = qkt[:]
                prev_qk = qk

            emit_gram(pend, NSLAB - 1)
            pend = None
            # norms for the last slab
            for m in range(3):
                nc.scalar.activation(
                    nsc.rearrange("p (r x) -> p r x", r=RSLAB)[:],
                    prev_qk[m][:],
                    mybir.ActivationFunctionType.Square,
                    accum_out=nrmp[m][:, NSLAB - 1:NSLAB])

            # ------------- softmax + A build + M^T -------------
            gs = [sm.tile([96, 96], F32, tag=f"gs{p}", name=f"gs{p}") for p in range(2)]
            for p in range(2):
                nc.scalar.copy(gs[p][:], g_t[p])
            p1.close()
            with tc.tile_pool(name="smps", bufs=2, space="PSUM") as smps, \
                 tc.tile_pool(name="mps", bufs=2, space="PSUM") as mps:
                sq = [sm.tile([96, 1], F32, tag=f"sq{p}", name=f"sq{p}") for p in range(2)]
                sk = [sm.tile([96, 1], F32, tag=f"sk{p}", name=f"sk{p}") for p in range(2)]
                # assemble per-pair norm sums from chunk partials
                # q: ch 0:192 = m0[0:128] + m1[0:64]; k: m1[64:128] + m2[0:128]
                # 32-partition pieces (quadrant rule: span<=32 unless 0-aligned)
                pieces = [(sq[0], 0, nrmp[0], 0), (sq[0], 32, nrmp[0], 32),
                          (sq[0], 64, nrmp[0], 64), (sq[1], 0, nrmp[0], 96),
                          (sq[1], 32, nrmp[1], 0), (sq[1], 64, nrmp[1], 32),
                          (sk[0], 0, nrmp[1], 64), (sk[0], 32, nrmp[1], 96),
                          (sk[0], 64, nrmp[2], 0), (sk[1], 0, nrmp[2], 32),
                          (sk[1], 32, nrmp[2], 64), (sk[1], 64, nrmp[2], 96)]
                for dst, do, srcp, so in pieces:
                    nc.vector.tensor_reduce(dst[do:do + 32, :], srcp[so:so + 32, :],
                                            op=ADD, axis=mybir.AxisListType.X)

                rq = [sm.tile([96, 1], F32, tag=f"rq{p}", name=f"rq{p}") for p in range(2)]
                rk = [sm.tile([96, 1], F32, tag=f"rk{p}", name=f"rk{p}") for p in range(2)]
                rqT = [sm.tile([1, 96], F32, tag=f"rqT{p}", name=f"rqT{p}") for p in range(2)]
                rkT = [sm.tile([1, 96], F32, tag=f"rkT{p}", name=f"rkT{p}") for p in range(2)]
                logit = [sm.tile([96, 96], F32, tag=f"lg{p}", name=f"lg{p}") for p in range(2)]
                nmax = [sm.tile([96, 1], F32, tag=f"nm{p}", name=f"nm{p}") for p in range(2)]
                ex = [sm.tile([96, 96], F32, tag=f"ex{p}", name=f"ex{p}") for p in range(2)]
                rs = [sm.tile([96, 1], F32, tag=f"rs{p}", name=f"rs{p}") for p in range(2)]
                aw = [sm.tile([96, 96], F16, tag=f"aw{p}", name=f"aw{p}") for p in range(2)]

                for p in range(2):   # 1/sqrt(s/temp) factors
                    nc.scalar.activation(rq[p][:], sq[p][:],
                                         mybir.ActivationFunctionType.Sqrt,
                                         scale=float(inv_temp))
                    nc.scalar.activation(rk[p][:], sk[p][:],
                                         mybir.ActivationFunctionType.Sqrt,
                                         scale=float(inv_temp))
                for p in range(2):
                    nc.vector.reciprocal(rq[p][:], rq[p][:])
                    nc.vector.reciprocal(rk[p][:], rk[p][:])
                for p in range(2):
                    tq = smps.tile([1, 96], F32, tag="rt", name=f"tq{p}")
                    nc.tensor.transpose(tq[:], rq[p][:], id32[:])
                    nc.vector.tensor_copy(rqT[p][:], tq[:])
                    tk = smps.tile([1, 96], F32, tag="rt", name=f"tk{p}")
                    nc.tensor.transpose(tk[:], rk[p][:], id32[:])
                    nc.vector.tensor_copy(rkT[p][:], tk[:])
                for p in range(2):   # logits = G * (rq x rk)
                    ops = smps.tile([96, 96], F32, tag="outer", name=f"op{p}")
                    nc.tensor.matmul(ops[:], rqT[p][0:1, :], rkT[p][0:1, :],
                                     start=True, stop=True)
                    nc.vector.tensor_mul(logit[p][:], gs[p][:], ops[:])
                for p in range(2):
                    nc.vector.reduce_max(nmax[p][:], logit[p][:], axis=mybir.AxisListType.X)
                    nc.vector.tensor_scalar_mul(nmax[p][:], nmax[p][:], -1.0)
                for p in range(2):
                    nc.scalar.activation(ex[p][:], logit[p][:],
                                         mybir.ActivationFunctionType.Exp,
                                         bias=nmax[p][:])
                for p in range(2):   # mask cross-head blocks, normalize rows
                    nc.vector.tensor_mul(ex[p][:], ex[p][:], bmask[:])
                    nc.vector.reduce_sum(rs[p][:], ex[p][:], axis=mybir.AxisListType.X)
                    nc.vector.reciprocal(rs[p][:], rs[p][:])
                    nc.vector.tensor_scalar_mul(aw[p][:], ex[p][:], rs[p][:])

                # A blocks (bf16): A1 = [aw0 | 0], A2 = [0 | aw1]  [96, 192]
                A1 = sm.tile([96, C], F16, tag="A1")
                A2 = sm.tile([96, C], F16, tag="A2")
                nc.gpsimd.memset(A1[:, 96:192], 0.0)
                nc.gpsimd.memset(A2[:, 0:96], 0.0)
                nc.vector.tensor_copy(A1[:, 0:96], aw[0][:])
                nc.vector.tensor_copy(A2[:, 96:192], aw[1][:])

                # M^T = A^T W_proj^T : [192 d, 192 o] in two partition chunks
                mt1p = mps.tile([128, C], F32, tag="mt1")
                nc.tensor.matmul(mt1p[:], A1[:, 0:128], wjlo[:], start=True, stop=False)
                nc.tensor.matmul(mt1p[:], A2[:, 0:128], wjhi[:], start=False, stop=True)
                mt2p = mps.tile([64, C], F32, tag="mt2")
                nc.tensor.matmul(mt2p[:], A1[:, 128:192], wjlo[:], start=True, stop=False)
                nc.tensor.matmul(mt2p[:], A2[:, 128:192], wjhi[:], start=False, stop=True)
                mt1 = sm.tile([128, C], F16, tag="mt1s")
                mt2 = sm.tile([64, C], F16, tag="mt2s")
                nc.vector.tensor_copy(mt1[:], mt1p[:])
                nc.vector.tensor_copy(mt2[:], mt2p[:])

            # ---------------- pass 2: out = M @ v (DMA from PSUM) ----------------
            with tc.tile_pool(name="po1", bufs=2, space="PSUM") as po1p, \
                 tc.tile_pool(name="po2", bufs=2, space="PSUM") as po2p, \
                 tc.tile_pool(name="osb", bufs=3) as osbp:
                for nt in range(N // 1024):
                    col = 1024 * nt
                    po1 = po1p.tile([128, 2, 512], F32, tag="po1", name=f"po1_{nt}")
                    po2 = po2p.tile([64, 2, 512], F32, tag="po2", name=f"po2_{nt}")
                    for h in range(2):
                        cc = col + 512 * h
                        nc.tensor.matmul(po1[:, h, :], mt1[:, 0:128],
                                         v16a[:, cc:cc + 512], start=True, stop=False)
                        nc.tensor.matmul(po1[:, h, :], mt2[:, 0:128],
                                         v16b[:, cc:cc + 512], start=False, stop=True)
                        nc.tensor.matmul(po2[:, h, :], mt1[:, 128:192],
                                         v16a[:, cc:cc + 512], start=True, stop=False)
                        nc.tensor.matmul(po2[:, h, :], mt2[:, 128:192],
                                         v16b[:, cc:cc + 512], start=False, stop=True)
                    o1 = osbp.tile([128, 1024], F32, tag="o1", name=f"o1_{nt}")
                    o2 = osbp.tile([64, 1024], F32, tag="o2", name=f"o2_{nt}")
                    if nt % 2 == 0:
                        nc.vector.tensor_copy(o1[:], po1.rearrange("p a b -> p (a b)"))
                        nc.scalar.copy(o2[:], po2.rearrange("p a b -> p (a b)"))
                    else:
                        nc.scalar.copy(o1[:], po1.rearrange("p a b -> p (a b)"))
                        nc.vector.tensor_copy(o2[:], po2.rearrange("p a b -> p (a b)"))
                    nc.sync.dma_start(out_d[0:128, col:col + 1024], o1[:])
                    nc.sync.dma_start(out_d[128:192, col:col + 1024], o2[:])

    nc.compile()
    return nc


def _host_inputs(x, w_pw, w_dw, w_proj):
    """Per-core DRAM input maps (weights shared across cores)."""
    f16 = ml_dtypes.bfloat16
    wpwT = np.ascontiguousarray(w_pw.T).astype(f16)        # [192, 576]
    wd9 = w_dw.reshape(C3, 9).astype(np.float32)
    wsc = np.zeros((128, NCH, 9), np.float32)
    for m in range(NCH):
        wsc[:, m, :] = wd9[128 * m:128 * m + 128, :]
    wdiag = np.zeros((128, 2, 9, 128), np.float32)
    for m in range(2):
        for t in range(9):
            wdiag[np.arange(128), m, t, np.arange(128)] = wd9[128 * m:128 * m + 128, t]
    wdiag4 = np.zeros((128, 9, 128), np.float32)
    for t in range(9):
        wdiag4[np.arange(128), t, np.arange(128)] = wd9[512 + (np.arange(128) % 64), t]
    wpjT = np.ascontiguousarray(w_proj.T)                  # [192 c, 192 o]
    shared = {
        "wpwa": wpwT[0:128],
        "wpwb": wpwT[128:192],
        "wsc": wsc,
        "wdiag": wdiag.astype(f16),
        "wdiag4": wdiag4.astype(f16),
        "wpjTlo": wpjT[0:96].astype(f16),
        "wpjThi": wpjT[96:192].astype(f16),
        "ident32": np.eye(96, dtype=np.float32),
        "bmask": np.kron(np.eye(2, dtype=np.float32), np.ones((48, 48), np.float32)),
    }
    maps = []
    for b in range(B):
        m = dict(shared)
        m["x16"] = x[b].astype(f16)
        maps.append(m)
    return maps


def kernel(x, w_pw, w_dw, w_proj, temperature, num_heads):
    x = np.asarray(x)
    w_pw = np.asarray(w_pw)
    w_dw = np.asarray(w_dw)
    w_proj = np.asarray(w_proj)
    temp = float(np.asarray(temperature))
    assert int(num_heads) == HEADS and x.shape == (B, C, HH, WW)

    key = ("prog", temp)
    if key not in _CACHE:
        _CACHE[key] = build_program(1.0 / temp)
    nc = _CACHE[key]

    in_maps = _host_inputs(x, w_pw, w_dw, w_proj)
    res = run_bass_kernel_spmd(nc, in_maps, core_ids=list(range(8)))
    out = np.stack([res.results[b]["out"].reshape(C, HH, WW) for b in range(B)])
    return out.astype(np.float32)


if __name__ == "__main__":
    rng = np.random.default_rng(0)
    x = rng.standard_normal((B, C, HH, WW), dtype=np.float32)
    w_pw = rng.standard_normal((C3, C), dtype=np.float32) * C ** -0.5
    w_dw = rng.standard_normal((C3, 1, 3, 3), dtype=np.float32) / 3.0
    w_proj = rng.standard_normal((C, C), dtype=np.float32) * C ** -0.5
    y = kernel(x, w_pw, w_dw, w_proj, np.float32((C / HEADS) ** -0.5), HEADS)
    print("out", y.shape, y.dtype, float(np.abs(y).max()))
